# revision 2
# baseline (speedup 1.0000x reference)
"""Dilated self-attention Trainium2 kernel.

Math: the reference runs 3 dilated-attention branches over x (b=4, n=8192,
c=128); every branch decomposes into independent causal attention problems of
identical shape (m=2048 tokens, d=128):
  branch (w=2048, r=1): 4 segments/batch, (w=4096, r=2): 2, (w=8192, r=4): 1
  -> 7 segments/batch x 4 batches = 28 identical tasks.

For each task the kernel computes the *unnormalized* attention
  U = (exp(S) * causal_mask) @ V @ Wo,   dsum = rowsum(exp(S) * causal_mask)
with S = (X Wq)(X Wk)^T / sqrt(c).  Since a branch's normalized output is
o = U/dsum with softmax denominator dsum, the cross-branch combine
  out[p] = sum_b o_b[p] * (dsum_b[p] / sum_b dsum_b[p]) = sum_b U_b[p] / sum_b dsum_b[p]
needs only U and dsum sums per position - no per-branch normalization.

Sharding: 28 tasks -> 8 cores x 4 segment slots (4 duplicated slots dropped on
the host).  Each core runs the same SPMD program on its own 4 segments.

On-core layout (per segment): everything is computed in the "transposed"
orientation so that no P-matrix transposes are ever needed:
  XT [c,2048]       shipped pre-transposed from the host
  QT = Wq~^T XT, KT = Wk^T XT          [d, 2048]   (Wq~ pre-scaled by 1/sqrt(c))
  V' = X W2 natural (W2 = Wv Wo host-folded)       [2048, c] as 16 [128,128] tiles
  per 512-query chunk cch, key tile j <= 4*cch+3 (diagonal tiles narrowed to
  their causally-live query columns and triangle-masked on GPSIMD):
    ST_j = KT_j^T QT_cch               [128 keys, 512 q]   (PSUM, f32)
    E_j  = exp(ST_j)  -> bf16 SBUF     (ACT engine; bf16 keeps all U/dsum
                                        matmuls at 1 cycle/row even when the
                                        diagonal narrows the free dim to 128,
                                        and halves SBUF traffic.  f16 would
                                        overflow: scores reach ~18, e^18>65504)
    dsum += ones^T E_j                 [1, 512]            (PSUM accum)
    U^T  += V'_j^T E_j                 [c, 512]            (PSUM accum)

The score->exp->accumulate chain is software-pipelined: score matmuls run
LOOKAHEAD tiles ahead of the U/dsum accumulation matmuls in the PE queue
(across chunk boundaries), so the PE never stalls waiting for ACT's exp.
Outputs per core: u [4, 128, 2048] (U^T) and d [4, 2048]; host transposes U.
"""

import sys

if "/opt/trn_rl_repo" not in sys.path:
    sys.path.insert(0, "/opt/trn_rl_repo")

import numpy as np

B, N, C = 4, 8192, 128
M = 2048                 # tokens per segment (same for every branch)
BRANCHES = [(2048, 1), (4096, 2), (8192, 4)]   # (w, r)
N_CORES = 8
SEGS_PER_CORE = 4        # 28 real segments + 4 duplicates
NT = M // 128            # 16 key/token tiles per segment
NCHUNK = M // 512        # 4 query chunks per segment
SCALE = 1.0 / np.sqrt(C)

_NC_CACHE = {}


def _segment_list():
    """All 28 (batch, w, r, seg_idx) tasks, in a fixed order."""
    segs = []
    for b in range(B):
        for (w, r) in BRANCHES:
            for t in range(N // w):
                segs.append((b, w, r, t))
    return segs


def _build_nc(loop_r=None):
    """Build the SPMD program. loop_r: if set, wrap the whole per-core body in
    a hardware For-loop with loop_r iterations (timing variant only)."""
    import contextlib

    import concourse.bass as bass
    import concourse.mybir as mybir
    import concourse.tile as tile
    from concourse import bacc
    from concourse.bass import ts

    f32 = mybir.dt.float32
    f32r = mybir.dt.float32r
    bf16 = mybir.dt.bfloat16
    f16 = mybir.dt.float16
    S = SEGS_PER_CORE

    nc = bacc.Bacc(None, target_bir_lowering=False)
    # x arrives pre-transposed (host-side): [S, C, M] = X^T per segment
    x_in = nc.dram_tensor("xseg", [S, C, M], f32r, kind="ExternalInput")
    xh_in = nc.dram_tensor("xsegh", [S, C, M], f16, kind="ExternalInput")
    wq_in = nc.dram_tensor("wq", [C, C], f32r, kind="ExternalInput")
    wk_in = nc.dram_tensor("wk", [C, C], f32r, kind="ExternalInput")
    # "wv" actually carries W2 = Wv @ Wo (host-folded), f16 for the narrow
    # V' projection
    wv_in = nc.dram_tensor("wv", [C, C], f16, kind="ExternalInput")
    msk_in = nc.dram_tensor("msk", [128, 128], f32, kind="ExternalInput")
    u_out = nc.dram_tensor("u", [S, C, M], f32, kind="ExternalOutput")
    d_out = nc.dram_tensor("d", [S, M], f32, kind="ExternalOutput")

    LA = 3                   # score-matmul lookahead (tiles) in the PE queue

    with tile.TileContext(nc) as tc:
        with (
            tc.tile_pool(name="const", bufs=1) as const,
            tc.tile_pool(name="xt", bufs=2) as xt_pool,
            tc.tile_pool(name="xh", bufs=2) as xh_pool,
            tc.tile_pool(name="qt", bufs=2) as qt_pool,
            tc.tile_pool(name="kt", bufs=2) as kt_pool,
            tc.tile_pool(name="vv", bufs=2) as v_pool,
            tc.tile_pool(name="ut", bufs=2) as ut_pool,
            tc.tile_pool(name="dd", bufs=2) as d_pool,
            tc.tile_pool(name="exp", bufs=8) as exp_pool,
            tc.tile_pool(name="psA", bufs=2, space="PSUM") as psA,        # transposes + projections
            tc.tile_pool(name="ps_s", bufs=4, space="PSUM") as ps_s_pool,  # scores
            tc.tile_pool(name="ps_u", bufs=1, space="PSUM") as ps_u_pool,  # O^T accumulator
            tc.tile_pool(name="ps_d", bufs=1, space="PSUM") as ps_d_pool,  # denominator accumulator
        ):
            wq_sb = const.tile([C, C], f32r)
            wk_sb = const.tile([C, C], f32r)
            wv_sb = const.tile([C, C], f16)
            nc.sync.dma_start(wq_sb[:], wq_in[:])
            nc.sync.dma_start(wk_sb[:], wk_in[:])
            nc.sync.dma_start(wv_sb[:], wv_in[:])
            msk_f = const.tile([128, 128], f32)
            nc.sync.dma_start(msk_f[:], msk_in[:])
            msk_sb = const.tile([128, 128], bf16)
            nc.vector.tensor_copy(msk_sb[:], msk_f[:])
            ones_f = const.tile([128, 1], f32)
            nc.vector.memset(ones_f[:], 1.0)
            ones_sb = const.tile([128, 1], bf16)
            nc.scalar.copy(out=ones_sb[:], in_=ones_f[:])
            loop_cm = (
                tc.For_i(0, loop_r, 1) if loop_r else contextlib.nullcontext()
            )
            with loop_cm:
              for s in range(S):
                # ---- stage 0: X^T arrives pre-transposed from the host
                xt = xt_pool.tile([C, M], f32r)
                nc.sync.dma_start(xt[:], x_in[s])
                xh = xh_pool.tile([C, M], f16)
                nc.sync.dma_start(xh[:], xh_in[s])

                # ---- stage 1: projections (Wq comes pre-scaled by 1/sqrt(c))
                qt = qt_pool.tile([C, M], f32r)
                kt = kt_pool.tile([C, M], f32r)
                for i in range(NCHUNK):
                    pq = psA.tile([128, 512], f32, tag="psA")
                    nc.tensor.matmul(pq[:], wq_sb[:], xt[:, ts(i, 512)])
                    nc.vector.tensor_copy(qt[:, ts(i, 512)], pq[:])
                    pk = psA.tile([128, 512], f32, tag="psA")
                    nc.tensor.matmul(pk[:], wk_sb[:], xt[:, ts(i, 512)])
                    nc.vector.tensor_copy(kt[:, ts(i, 512)], pk[:])
                v_sb = v_pool.tile([128, NT, C], bf16)
                for g in range(NT // 4):
                    # 4 V-projection matmuls into one PSUM tile -> one copy
                    pv = psA.tile([128, 512], f32, tag="psA")
                    for t4 in range(4):
                        nc.tensor.matmul(
                            pv[:, ts(t4, 128)],
                            xh[:, ts(4 * g + t4, 128)],
                            wv_sb[:],
                        )
                    nc.vector.tensor_copy(
                        v_sb[:, 4 * g : 4 * g + 4, :].rearrange("p t c -> p (t c)"),
                        pv[:],
                    )

                # ---- stage 2: attention, software-pipelined over all
                # (chunk, key-tile) tasks of the segment.  V already carries
                # Wo (host-folded W2 = Wv @ Wo), so ps_u accumulates U^T
                # directly and the per-chunk epilogue is just two copies.
                ut = ut_pool.tile([C, M], f32)
                d_sb = d_pool.tile([1, M], f32)

                tasks = []
                for cch in range(NCHUNK):
                    # diagonal tiles first: their mask op overlaps the full
                    # tiles' matmuls instead of stalling the accumulation
                    js = list(range(4 * cch, 4 * cch + 4)) + list(range(0, 4 * cch))
                    for pos, j in enumerate(js):
                        tasks.append((cch, j, pos == 0, pos == len(js) - 1))

                n_tasks = len(tasks)
                e_state = {}
                psu_state = {}

                def emit_score(t):
                    cch, j, _, _ = tasks[t]
                    # diagonal tiles only touch queries >= their key offset
                    lo = 128 * (j - 4 * cch) if j >= 4 * cch else 0
                    ps_sc = ps_s_pool.tile([128, 512], f32, tag="ps_s",
                                           name="ps_sc")
                    nc.tensor.matmul(
                        ps_sc[:, lo:512],
                        kt[:, ts(j, 128)],
                        qt[:, cch * 512 + lo : (cch + 1) * 512],
                    )
                    e = exp_pool.tile([128, 512], bf16, name="e")
                    nc.scalar.activation(
                        out=e[:, lo:512], in_=ps_sc[:, lo:512],
                        func=mybir.ActivationFunctionType.Exp,
                    )
                    if j >= 4 * cch:
                        # SBUF-only op -> GPSIMD, keeping DVE free for copies
                        nc.gpsimd.tensor_mul(
                            out=e[:, lo : lo + 128],
                            in0=e[:, lo : lo + 128],
                            in1=msk_sb[:],
                        )
                    e_state[t] = (e, lo)

                def emit_accum(t):
                    cch, j, first, last = tasks[t]
                    e, lo = e_state.pop(t)
                    if first:
                        psu_state[cch] = (
                            ps_u_pool.tile([128, 512], f32, name="ps_u"),
                            ps_d_pool.tile([1, 512], f32, name="ps_d"),
                        )
                    ps_u, ps_d = psu_state[cch]
                    nc.tensor.matmul(
                        ps_u[:, lo:512], v_sb[:, j, :], e[:, lo:512],
                        start=first, stop=last,
                    )
                    nc.tensor.matmul(
                        ps_d[:, lo:512], ones_sb[:], e[:, lo:512],
                        start=first, stop=last,
                    )
                    if last:
                        nc.vector.tensor_copy(d_sb[:, ts(cch, 512)], ps_d[:])
                        nc.vector.tensor_copy(ut[:, ts(cch, 512)], ps_u[:])

                for t in range(n_tasks + LA):
                    if t < n_tasks:
                        emit_score(t)
                    if t >= LA:
                        emit_accum(t - LA)

                nc.sync.dma_start(u_out[s], ut[:])
                nc.sync.dma_start(d_out[s : s + 1, :], d_sb[:])

    nc.compile()
    return nc


def get_nc(loop_r=None):
    key = ("nc", loop_r)
    if key not in _NC_CACHE:
        _NC_CACHE[key] = _build_nc(loop_r)
    return _NC_CACHE[key]


def _masks():
    """Diagonal-block triangle: msk[kk, qq] = 1.0 iff kk <= qq."""
    kk = np.arange(128)[:, None]
    qq = np.arange(128)[None, :]
    return (kk <= qq).astype(np.float32)


def build_in_maps(x, Wq, Wk, Wv, Wo):
    segs = _segment_list()
    padded = segs + segs[:N_CORES * SEGS_PER_CORE - len(segs)]
    msk = _masks()
    in_maps = []
    for core in range(N_CORES):
        xseg = np.empty((SEGS_PER_CORE, C, M), dtype=np.float32)
        for k in range(SEGS_PER_CORE):
            b, w, r, t = padded[core * SEGS_PER_CORE + k]
            xseg[k] = x[b, t * w + r * np.arange(M), :].T
        in_maps.append({
            "xseg": xseg,
            "xsegh": xseg.astype(np.float16),
            # 1/sqrt(c) score scaling folded into Wq on the host
            "wq": np.ascontiguousarray(Wq, dtype=np.float32) * np.float32(SCALE),
            "wk": np.ascontiguousarray(Wk, dtype=np.float32),
            # W2 = Wv @ Wo folded on the host; Wo never ships to the device
            "wv": (np.asarray(Wv, dtype=np.float64) @ np.asarray(Wo, dtype=np.float64)).astype(np.float16),
            "msk": msk,
        })
    return in_maps, padded


def combine(results, padded):
    """results: per-core dicts with u [S,C,M] and d [S,M]."""
    numer = np.zeros((B, N, C), dtype=np.float64)
    den = np.zeros((B, N), dtype=np.float64)
    seen = set()
    for core in range(N_CORES):
        for k in range(SEGS_PER_CORE):
            key = padded[core * SEGS_PER_CORE + k]
            if key in seen:
                continue
            seen.add(key)
            b, w, r, t = key
            pos = t * w + r * np.arange(M)
            numer[b, pos, :] += results[core]["u"][k].T.astype(np.float64)
            den[b, pos] += results[core]["d"][k].astype(np.float64)
    return (numer / den[..., None]).astype(np.float32)


def kernel(x, Wq, Wk, Wv, Wo):
    from concourse.bass_utils import run_bass_kernel_spmd

    x = np.asarray(x, dtype=np.float32)
    nc = get_nc()
    in_maps, padded = build_in_maps(x, Wq, Wk, Wv, Wo)
    res = run_bass_kernel_spmd(nc, in_maps, core_ids=list(range(N_CORES)))
    return combine(res.results, padded)


if __name__ == "__main__":
    rng = np.random.default_rng(0)
    x = rng.standard_normal((B, N, C)).astype(np.float32)
    Wq, Wk, Wv, Wo = [
        (rng.standard_normal((C, C)) / np.sqrt(C)).astype(np.float32)
        for _ in range(4)
    ]
    out = kernel(x, Wq, Wk, Wv, Wo)
    print("out", out.shape, out.dtype, np.abs(out).max())


# revision 7
# speedup vs baseline: 1.0519x; 1.0519x over previous
"""Dilated self-attention Trainium2 kernel.

Math: the reference runs 3 dilated-attention branches over x (b=4, n=8192,
c=128); every branch decomposes into independent causal attention problems of
identical shape (m=2048 tokens, d=128):
  branch (w=2048, r=1): 4 segments/batch, (w=4096, r=2): 2, (w=8192, r=4): 1
  -> 7 segments/batch x 4 batches = 28 identical tasks.

For each task the kernel computes the *unnormalized* attention
  U = (exp(S) * causal_mask) @ V @ Wo,   dsum = rowsum(exp(S) * causal_mask)
with S = (X Wq)(X Wk)^T / sqrt(c).  The cross-branch combine
  out[p] = sum_b U_b[p] / sum_b dsum_b[p]
needs only U and dsum sums per position - no per-branch normalization.

Sharding: 28 tasks -> 8 cores x 4 segment slots (4 duplicated slots dropped on
the host).  Each core runs the same SPMD program on its own 4 segments.

On-core layout (per segment), transposed orientation (no transposes needed):
  XT [c,2048]  shipped pre-transposed; S = X G X^T with G = (Wq/sqrt(c)) Wk^T
  host-folded, so only ONE projection feeds the scores:
    PT = G^T XT                        [c, 2048]
    ST_j = XT_j^T PT_cch               [128 keys, 512 q]   (PSUM f32)
  V' = X W2 natural (W2 = Wv Wo host-folded)  [2048, c] as 16 [128,128] tiles
  E_j = exp(ST_j) -> bf16 SBUF (ACT; bf16 keeps every matmul at 1 cycle/row
  and halves traffic; f16 would overflow: scores reach ~18, e^18 > 65504).
  Scores/exp are emitted in PAIRS sharing a 2-bank PSUM tile so non-diagonal
  exps batch two tiles per ACT instruction.
    U^T  += V'_j^T E_j                 [c, 512]            (PSUM accum)
  dsum: diagonal tiles matmul ones^T E_j directly (narrowed); full quads of 4
  E tiles are pre-summed elementwise (2 Pool adds + 1 DVE add) so one
  [1,512] matmul per QUAD replaces four - cutting the PE's dsum work ~2x.
  Chunk results DMA straight from PSUM to DRAM (no SBUF staging copies).

The score->exp->accumulate chain is software-pipelined at pair granularity
(scores run 2 pairs ahead of accumulation in the PE queue).
Outputs per core: u [4, 128, 2048] (U^T) and d [4, 2048]; host transposes U.
"""

import sys

if "/opt/trn_rl_repo" not in sys.path:
    sys.path.insert(0, "/opt/trn_rl_repo")

import numpy as np

B, N, C = 4, 8192, 128
M = 2048                 # tokens per segment (same for every branch)
BRANCHES = [(2048, 1), (4096, 2), (8192, 4)]   # (w, r)
N_CORES = 8
SEGS_PER_CORE = 4        # 28 real segments + 4 duplicates
NT = M // 128            # 16 key/token tiles per segment
NCHUNK = M // 512        # 4 query chunks per segment
SCALE = 1.0 / np.sqrt(C)

_NC_CACHE = {}


def _segment_list():
    """All 28 (batch, w, r, seg_idx) tasks, in a fixed order."""
    segs = []
    for b in range(B):
        for (w, r) in BRANCHES:
            for t in range(N // w):
                segs.append((b, w, r, t))
    return segs


def _build_nc(loop_r=None):
    """Build the SPMD program. loop_r: if set, wrap the whole per-core body in
    a hardware For-loop with loop_r iterations (timing variant only)."""
    import contextlib

    import concourse.bass as bass
    import concourse.mybir as mybir
    import concourse.tile as tile
    from concourse import bacc
    from concourse.bass import ts

    f32 = mybir.dt.float32
    f32r = mybir.dt.float32r
    bf16 = mybir.dt.bfloat16
    f16 = mybir.dt.float16
    S = SEGS_PER_CORE

    nc = bacc.Bacc(None, target_bir_lowering=False)
    # x arrives pre-transposed (host-side): [S, C, M] = X^T per segment
    x_in = nc.dram_tensor("xseg", [S, C, M], f32r, kind="ExternalInput")
    xh_in = nc.dram_tensor("xsegh", [S, C, M], f16, kind="ExternalInput")
    # "g" = (Wq/sqrt(c)) @ Wk^T host-folded: S = X G X^T
    g_in = nc.dram_tensor("g", [C, C], f32r, kind="ExternalInput")
    # "wv" actually carries W2 = Wv @ Wo (host-folded)
    wv_in = nc.dram_tensor("wv", [C, C], f16, kind="ExternalInput")
    msk_in = nc.dram_tensor("msk", [128, 128], f32, kind="ExternalInput")
    u_out = nc.dram_tensor("u", [S, C, M], f32, kind="ExternalOutput")
    d_out = nc.dram_tensor("d", [S, M], f32, kind="ExternalOutput")

    LA = 2                   # score lookahead in PAIRS (2 tiles each)

    with tile.TileContext(nc) as tc:
        with (
            tc.tile_pool(name="const", bufs=1) as const,
            tc.tile_pool(name="xt", bufs=2) as xt_pool,
            tc.tile_pool(name="xh", bufs=2) as xh_pool,
            tc.tile_pool(name="pt", bufs=2) as pt_pool,
            tc.tile_pool(name="vv", bufs=2) as v_pool,
            tc.tile_pool(name="exp", bufs=6) as exp_pool,
            tc.tile_pool(name="hsum", bufs=3) as hs_pool,
            tc.tile_pool(name="qsum", bufs=2) as q_pool,
            tc.tile_pool(name="ut", bufs=2) as ut_pool,
            tc.tile_pool(name="dd", bufs=2) as d_pool,
            tc.tile_pool(name="psS", bufs=3, space="PSUM") as psS,         # 2-bank score/proj megas
            tc.tile_pool(name="ps_u", bufs=1, space="PSUM") as ps_u_pool,  # U^T accumulator
            tc.tile_pool(name="ps_d", bufs=1, space="PSUM") as ps_d_pool,  # denominator accumulator
        ):
            g_sb = const.tile([C, C], f32r)
            wv_sb = const.tile([C, C], f16)
            nc.sync.dma_start(g_sb[:], g_in[:])
            nc.sync.dma_start(wv_sb[:], wv_in[:])
            msk_f = const.tile([128, 128], f32)
            nc.sync.dma_start(msk_f[:], msk_in[:])
            msk_sb = const.tile([128, 128], bf16)
            nc.vector.tensor_copy(msk_sb[:], msk_f[:])
            ones_f = const.tile([128, 1], f32)
            nc.vector.memset(ones_f[:], 1.0)
            ones_sb = const.tile([128, 1], bf16)
            nc.scalar.copy(out=ones_sb[:], in_=ones_f[:])
            loop_cm = (
                tc.For_i(0, loop_r, 1) if loop_r else contextlib.nullcontext()
            )
            with loop_cm:
              for s in range(S):
                # ---- stage 0: X^T arrives pre-transposed from the host
                xt = xt_pool.tile([C, M], f32r)
                nc.sync.dma_start(xt[:], x_in[s])
                xh = xh_pool.tile([C, M], f16)
                nc.sync.dma_start(xh[:], xh_in[s])

                # ---- stage 1: projections.  P = G^T X^T feeds the scores
                # (S^T_j = X^T_j^T P); V' = X W2 natural.
                pt = pt_pool.tile([C, M], f32r)
                for m in range(2):
                    pm = psS.tile([128, 2, 512], f32, tag="s", name="pm")
                    for h in range(2):
                        nc.tensor.matmul(
                            pm[:, h, :], g_sb[:], xt[:, ts(2 * m + h, 512)]
                        )
                    nc.vector.tensor_copy(
                        pt[:, ts(m, 1024)], pm.rearrange("p a b -> p (a b)")
                    )
                v_sb = v_pool.tile([128, NT, C], bf16)
                for m in range(2):
                    vm = psS.tile([128, 2, 512], f32, tag="s", name="vm")
                    for t8 in range(8):
                        nc.tensor.matmul(
                            vm[:, t8 // 4, ts(t8 % 4, 128)],
                            xh[:, ts(8 * m + t8, 128)],
                            wv_sb[:],
                        )
                    nc.vector.tensor_copy(
                        v_sb[:, 8 * m : 8 * m + 8, :].rearrange("p t c -> p (t c)"),
                        vm.rearrange("p a b -> p (a b)"),
                    )

                # ---- stage 2: attention, software-pipelined over tile PAIRS.
                # Diagonal tiles are PACKED: their score matmuls write at
                # shifted column offsets so each diagonal mega is one
                # contiguous region -> one exp per mega (2 per chunk, not 4).
                # Matmul moving-operand columns map to output columns by
                # position, so a shifted E slice feeds U/dsum unchanged.
                #   mega D1: t0 at flat [0:512],  t1 at flat [512:896]
                #   mega D2: t2 at flat [0:256],  t3 at flat [256:384]
                # dsum: the 4 ragged diagonal tiles are combined with
                # column-ALIGNED slice adds into one [128,512] quad (Pool),
                # so each chunk needs 1 diagonal d-matmul + 1 per full quad.
                ut = ut_pool.tile([C, M], f32)
                d_sb = d_pool.tile([1, M], f32)
                # task kinds: "D1"/"D2" diagonal megas, "F" full pair
                pairs = []   # (cch, kind, j0, j1, first, last)
                for cch in range(NCHUNK):
                    pl = [("D1", 4 * cch, 4 * cch + 1),
                          ("D2", 4 * cch + 2, 4 * cch + 3)]
                    pl += [("F", 2 * i, 2 * i + 1) for i in range(2 * cch)]
                    for k, (kind, a, b) in enumerate(pl):
                        pairs.append((cch, kind, a, b, k == 0, k == len(pl) - 1))

                n_pairs = len(pairs)
                e_state = {}
                chunk_state = {}

                def emit_score(p):
                    cch, kind, j0, j1, _, _ = pairs[p]
                    q0 = cch * 512
                    sm = psS.tile([128, 2, 512], f32, tag="s", name="sm")
                    e = exp_pool.tile([128, 2, 512], bf16, name="e")
                    ef = e.rearrange("p a b -> p (a b)")
                    smf = sm.rearrange("p a b -> p (a b)")
                    if kind == "D1":
                        # t0: q [0:512) at flat [0:512); t1: q [128:512) at flat [512:896)
                        nc.tensor.matmul(smf[:, 0:512], xt[:, ts(j0, 128)],
                                         pt[:, q0 : q0 + 512])
                        nc.tensor.matmul(smf[:, 512:896], xt[:, ts(j1, 128)],
                                         pt[:, q0 + 128 : q0 + 512])
                        nc.scalar.activation(
                            out=ef[:, 0:896], in_=smf[:, 0:896],
                            func=mybir.ActivationFunctionType.Exp,
                        )
                        nc.gpsimd.tensor_mul(out=ef[:, 0:128],
                                             in0=ef[:, 0:128], in1=msk_sb[:])
                        nc.gpsimd.tensor_mul(out=ef[:, 512:640],
                                             in0=ef[:, 512:640], in1=msk_sb[:])
                    elif kind == "D2":
                        # t2: q [256:512) at flat [0:256); t3: q [384:512) at flat [256:384)
                        nc.tensor.matmul(smf[:, 0:256], xt[:, ts(j0, 128)],
                                         pt[:, q0 + 256 : q0 + 512])
                        nc.tensor.matmul(smf[:, 256:384], xt[:, ts(j1, 128)],
                                         pt[:, q0 + 384 : q0 + 512])
                        nc.scalar.activation(
                            out=ef[:, 0:384], in_=smf[:, 0:384],
                            func=mybir.ActivationFunctionType.Exp,
                        )
                        nc.gpsimd.tensor_mul(out=ef[:, 0:128],
                                             in0=ef[:, 0:128], in1=msk_sb[:])
                        nc.gpsimd.tensor_mul(out=ef[:, 256:384],
                                             in0=ef[:, 256:384], in1=msk_sb[:])
                    else:
                        for h, j in enumerate((j0, j1)):
                            nc.tensor.matmul(sm[:, h, :], xt[:, ts(j, 128)],
                                             pt[:, q0 : q0 + 512])
                        nc.scalar.activation(
                            out=ef[:], in_=smf[:],
                            func=mybir.ActivationFunctionType.Exp,
                        )
                        # half-sum for the dsum quad (Pool, overlaps PE)
                        hs = hs_pool.tile([128, 512], bf16, name="hs")
                        nc.gpsimd.tensor_add(hs[:], e[:, 0, :], e[:, 1, :])
                        e_state[("hs", p)] = hs
                    e_state[p] = e

                def emit_accum(p):
                    cch, kind, j0, j1, first, last = pairs[p]
                    e = e_state.pop(p)
                    ef = e.rearrange("p a b -> p (a b)")
                    if first:
                        chunk_state[cch] = {
                            "u": ps_u_pool.tile([128, 512], f32, name="ps_u"),
                            "d": ps_d_pool.tile([1, 512], f32, name="ps_d"),
                            "nd": 0,          # d-matmuls emitted (of 1 + cch)
                            "hs": None,       # pending half-sum for full quad
                            "e1": None,       # D1's e, consumed by D2's combine
                        }
                    st = chunk_state[cch]
                    ps_u, ps_d = st["u"], st["d"]
                    n_d = 1 + cch
                    if kind == "D1":
                        nc.tensor.matmul(ps_u[:, 0:512], v_sb[:, j0, :],
                                         ef[:, 0:512], start=True, stop=False)
                        nc.tensor.matmul(ps_u[:, 128:512], v_sb[:, j1, :],
                                         ef[:, 512:896], start=False,
                                         stop=False)
                        st["e1"] = e
                    elif kind == "D2":
                        stop_u = (cch == 0)
                        nc.tensor.matmul(ps_u[:, 256:512], v_sb[:, j0, :],
                                         ef[:, 0:256], start=False, stop=False)
                        nc.tensor.matmul(ps_u[:, 384:512], v_sb[:, j1, :],
                                         ef[:, 256:384], start=False,
                                         stop=stop_u)
                        # column-aligned ragged combine of the 4 diag tiles
                        e1 = st.pop("e1").rearrange("p a b -> p (a b)")
                        qd = q_pool.tile([128, 512], bf16, name="qd")
                        nc.gpsimd.tensor_copy(qd[:, 0:128], e1[:, 0:128])
                        nc.gpsimd.tensor_add(qd[:, 128:512], e1[:, 128:512],
                                             e1[:, 512:896])
                        nc.gpsimd.tensor_add(qd[:, 256:512], qd[:, 256:512],
                                             ef[:, 0:256])
                        nc.gpsimd.tensor_add(qd[:, 384:512], qd[:, 384:512],
                                             ef[:, 256:384])
                        nc.tensor.matmul(ps_d[:, 0:512], ones_sb[:], qd[:],
                                         start=True, stop=(cch == 0))
                        st["nd"] = 1
                    else:
                        for h, j in enumerate((j0, j1)):
                            nc.tensor.matmul(
                                ps_u[:, 0:512], v_sb[:, j, :], e[:, h, :],
                                start=False, stop=(last and h == 1),
                            )
                        hs = e_state.pop(("hs", p))
                        if st["hs"] is None:
                            st["hs"] = hs
                        else:
                            qd = q_pool.tile([128, 512], bf16, name="qd")
                            nc.vector.tensor_add(qd[:], st["hs"][:], hs[:])
                            st["hs"] = None
                            nc.tensor.matmul(
                                ps_d[:, 0:512], ones_sb[:], qd[:],
                                start=False, stop=(st["nd"] == n_d - 1),
                            )
                            st["nd"] += 1
                    if last:
                        # stage through SBUF (DMA cannot read PSUM); u leaves
                        # chunk-wise so the final drain is short, d once/segment
                        nc.vector.tensor_copy(ut[:, ts(cch, 512)], ps_u[:])
                        nc.vector.tensor_copy(d_sb[:, ts(cch, 512)], ps_d[:])
                        nc.sync.dma_start(
                            u_out[s, :, 512 * cch : 512 * (cch + 1)],
                            ut[:, ts(cch, 512)],
                        )

                for p in range(n_pairs + LA):
                    if p < n_pairs:
                        emit_score(p)
                    if p >= LA:
                        emit_accum(p - LA)
                nc.sync.dma_start(d_out[s : s + 1, :], d_sb[:])

    nc.compile()
    return nc


def get_nc(loop_r=None):
    key = ("nc", loop_r)
    if key not in _NC_CACHE:
        _NC_CACHE[key] = _build_nc(loop_r)
    return _NC_CACHE[key]


def _masks():
    """Diagonal-block triangle: msk[kk, qq] = 1.0 iff kk <= qq."""
    kk = np.arange(128)[:, None]
    qq = np.arange(128)[None, :]
    return (kk <= qq).astype(np.float32)


def build_in_maps(x, Wq, Wk, Wv, Wo):
    segs = _segment_list()
    padded = segs + segs[:N_CORES * SEGS_PER_CORE - len(segs)]
    msk = _masks()
    Wq64 = np.asarray(Wq, dtype=np.float64)
    Wk64 = np.asarray(Wk, dtype=np.float64)
    in_maps = []
    for core in range(N_CORES):
        xseg = np.empty((SEGS_PER_CORE, C, M), dtype=np.float32)
        for k in range(SEGS_PER_CORE):
            b, w, r, t = padded[core * SEGS_PER_CORE + k]
            xseg[k] = x[b, t * w + r * np.arange(M), :].T
        in_maps.append({
            "xseg": xseg,
            "xsegh": xseg.astype(np.float16),
            # G = (Wq/sqrt(c)) Wk^T folded on the host: S = X G X^T
            "g": (Wq64 @ Wk64.T).astype(np.float32) * np.float32(SCALE),
            # W2 = Wv @ Wo folded on the host; Wo never ships to the device
            "wv": (np.asarray(Wv, dtype=np.float64) @ np.asarray(Wo, dtype=np.float64)).astype(np.float16),
            "msk": msk,
        })
    return in_maps, padded


def combine(results, padded):
    """results: per-core dicts with u [S,C,M] and d [S,M]."""
    numer = np.zeros((B, N, C), dtype=np.float64)
    den = np.zeros((B, N), dtype=np.float64)
    seen = set()
    for core in range(N_CORES):
        for k in range(SEGS_PER_CORE):
            key = padded[core * SEGS_PER_CORE + k]
            if key in seen:
                continue
            seen.add(key)
            b, w, r, t = key
            pos = t * w + r * np.arange(M)
            numer[b, pos, :] += results[core]["u"][k].T.astype(np.float64)
            den[b, pos] += results[core]["d"][k].astype(np.float64)
    return (numer / den[..., None]).astype(np.float32)


def kernel(x, Wq, Wk, Wv, Wo):
    from concourse.bass_utils import run_bass_kernel_spmd

    x = np.asarray(x, dtype=np.float32)
    nc = get_nc()
    in_maps, padded = build_in_maps(x, Wq, Wk, Wv, Wo)
    res = run_bass_kernel_spmd(nc, in_maps, core_ids=list(range(N_CORES)))
    return combine(res.results, padded)


if __name__ == "__main__":
    rng = np.random.default_rng(0)
    x = rng.standard_normal((B, N, C)).astype(np.float32)
    Wq, Wk, Wv, Wo = [
        (rng.standard_normal((C, C)) / np.sqrt(C)).astype(np.float32)
        for _ in range(4)
    ]
    out = kernel(x, Wq, Wk, Wv, Wo)
    print("out", out.shape, out.dtype, np.abs(out).max())


# revision 9
# speedup vs baseline: 1.1882x; 1.1297x over previous
"""Dilated self-attention Trainium2 kernel.

Math: the reference runs 3 dilated-attention branches over x (b=4, n=8192,
c=128); every branch decomposes into independent causal attention problems of
identical shape (m=2048 tokens, d=128):
  branch (w=2048, r=1): 4 segments/batch, (w=4096, r=2): 2, (w=8192, r=4): 1
  -> 7 segments/batch x 4 batches = 28 identical tasks.

For each task the kernel computes the *unnormalized* attention
  U = (exp(S) * causal_mask) @ V @ Wo,   dsum = rowsum(exp(S) * causal_mask)
with S = (X Wq)(X Wk)^T / sqrt(c).  The cross-branch combine
  out[p] = sum_b U_b[p] / sum_b dsum_b[p]
needs only U and dsum sums per position - no per-branch normalization.

Sharding: 28 tasks -> 8 cores x 4 segment slots (4 duplicated slots dropped on
the host).  Each core runs the same SPMD program on its own 4 segments.

On-core layout (per segment), transposed orientation (no transposes needed):
  XT [c,2048]  shipped pre-transposed; S = X G X^T with G = (Wq/sqrt(c)) Wk^T
  host-folded, so only ONE projection feeds the scores:
    PT = G^T XT                        [c, 2048]
    ST_j = XT_j^T PT_cch               [128 keys, 512 q]   (PSUM f32)
  V' = X W2 natural (W2 = Wv Wo host-folded)  [2048, c] as 16 [128,128] tiles
  E_j = exp(ST_j) -> bf16 SBUF (ACT; bf16 keeps every matmul at 1 cycle/row
  and halves traffic; f16 would overflow: scores reach ~18, e^18 > 65504).
  Scores/exp are emitted in PAIRS sharing a 2-bank PSUM tile so non-diagonal
  exps batch two tiles per ACT instruction.
    U^T  += V'_j^T E_j                 [c, 512]            (PSUM accum)
  dsum: diagonal tiles matmul ones^T E_j directly (narrowed); full quads of 4
  E tiles are pre-summed elementwise (2 Pool adds + 1 DVE add) so one
  [1,512] matmul per QUAD replaces four - cutting the PE's dsum work ~2x.
  Chunk results DMA straight from PSUM to DRAM (no SBUF staging copies).

The score->exp->accumulate chain is software-pipelined at pair granularity
(scores run 2 pairs ahead of accumulation in the PE queue).
Outputs per core: u [4, 128, 2048] (U^T) and d [4, 2048]; host transposes U.
"""

import sys

if "/opt/trn_rl_repo" not in sys.path:
    sys.path.insert(0, "/opt/trn_rl_repo")

import numpy as np

B, N, C = 4, 8192, 128
M = 2048                 # tokens per segment (same for every branch)
BRANCHES = [(2048, 1), (4096, 2), (8192, 4)]   # (w, r)
N_CORES = 8
SEGS_PER_CORE = 4        # 28 real segments + 4 duplicates
NT = M // 128            # 16 key/token tiles per segment
NCHUNK = M // 512        # 4 query chunks per segment
SCALE = 1.0 / np.sqrt(C)

_NC_CACHE = {}


def _segment_list():
    """All 28 (batch, w, r, seg_idx) tasks, in a fixed order."""
    segs = []
    for b in range(B):
        for (w, r) in BRANCHES:
            for t in range(N // w):
                segs.append((b, w, r, t))
    return segs


def _build_nc(loop_r=None):
    """Build the SPMD program. loop_r: if set, wrap the whole per-core body in
    a hardware For-loop with loop_r iterations (timing variant only)."""
    import contextlib

    import concourse.bass as bass
    import concourse.mybir as mybir
    import concourse.tile as tile
    from concourse import bacc
    from concourse.bass import ts

    f32 = mybir.dt.float32
    f32r = mybir.dt.float32r
    bf16 = mybir.dt.bfloat16
    f16 = mybir.dt.float16
    S = SEGS_PER_CORE

    nc = bacc.Bacc(None, target_bir_lowering=False)
    # x arrives pre-transposed (host-side): [S, C, M] = X^T per segment
    x_in = nc.dram_tensor("xseg", [S, C, M], f32r, kind="ExternalInput")
    xh_in = nc.dram_tensor("xsegh", [S, C, M], f16, kind="ExternalInput")
    # "g" = (Wq/sqrt(c)) @ Wk^T host-folded: S = X G X^T
    g_in = nc.dram_tensor("g", [C, C], f32r, kind="ExternalInput")
    # "wv" actually carries W2 = Wv @ Wo (host-folded)
    wv_in = nc.dram_tensor("wv", [C, C], f16, kind="ExternalInput")
    msk_in = nc.dram_tensor("msk", [128, 128], f32, kind="ExternalInput")
    u_out = nc.dram_tensor("u", [S, C, M], f32, kind="ExternalOutput")
    d_out = nc.dram_tensor("d", [S, M], f32, kind="ExternalOutput")

    LA = 2                   # score lookahead in PAIRS (2 tiles each)

    with tile.TileContext(nc) as tc:
        with (
            tc.tile_pool(name="const", bufs=1) as const,
            tc.tile_pool(name="xt", bufs=2) as xt_pool,
            tc.tile_pool(name="xh", bufs=2) as xh_pool,
            tc.tile_pool(name="pt", bufs=2) as pt_pool,
            tc.tile_pool(name="vv", bufs=2) as v_pool,
            tc.tile_pool(name="exp", bufs=10) as exp_pool,
            tc.tile_pool(name="ut", bufs=2) as ut_pool,
            tc.tile_pool(name="dd", bufs=2) as d_pool,
            tc.tile_pool(name="psS", bufs=3, space="PSUM") as psS,         # 2-bank score/proj megas
            tc.tile_pool(name="ps_u", bufs=1, space="PSUM") as ps_u_pool,  # U^T accumulator
            tc.tile_pool(name="ps_d", bufs=1, space="PSUM") as ps_d_pool,  # denominator accumulator
        ):
            g_sb = const.tile([C, C], f32r)
            wv_sb = const.tile([C, C], f16)
            nc.sync.dma_start(g_sb[:], g_in[:])
            nc.sync.dma_start(wv_sb[:], wv_in[:])
            msk_f = const.tile([128, 128], f32)
            nc.sync.dma_start(msk_f[:], msk_in[:])
            msk_sb = const.tile([128, 128], bf16)
            nc.vector.tensor_copy(msk_sb[:], msk_f[:])
            ones_f = const.tile([128, 1], f32)
            nc.vector.memset(ones_f[:], 1.0)
            ones_sb = const.tile([128, 1], bf16)
            nc.scalar.copy(out=ones_sb[:], in_=ones_f[:])
            loop_cm = (
                tc.For_i(0, loop_r, 1) if loop_r else contextlib.nullcontext()
            )
            with loop_cm:
              for s in range(S):
                # ---- stage 0: X^T arrives pre-transposed from the host
                xt = xt_pool.tile([C, M], f32r)
                nc.sync.dma_start(xt[:], x_in[s])
                xh = xh_pool.tile([C, M], f16)
                nc.sync.dma_start(xh[:], xh_in[s])

                # ---- stage 1: projections.  P = G^T X^T feeds the scores
                # (S^T_j = X^T_j^T P); V' = X W2 natural.
                pt = pt_pool.tile([C, M], f32r)
                for m in range(2):
                    pm = psS.tile([128, 2, 512], f32, tag="s", name="pm")
                    for h in range(2):
                        nc.tensor.matmul(
                            pm[:, h, :], g_sb[:], xt[:, ts(2 * m + h, 512)]
                        )
                    nc.vector.tensor_copy(
                        pt[:, ts(m, 1024)], pm.rearrange("p a b -> p (a b)")
                    )
                v_sb = v_pool.tile([128, NT, C], bf16)
                for m in range(2):
                    vm = psS.tile([128, 2, 512], f32, tag="s", name="vm")
                    for t8 in range(8):
                        nc.tensor.matmul(
                            vm[:, t8 // 4, ts(t8 % 4, 128)],
                            xh[:, ts(8 * m + t8, 128)],
                            wv_sb[:],
                        )
                    nc.vector.tensor_copy(
                        v_sb[:, 8 * m : 8 * m + 8, :].rearrange("p t c -> p (t c)"),
                        vm.rearrange("p a b -> p (a b)"),
                    )

                # ---- stage 2: attention, software-pipelined over tile PAIRS.
                # Diagonal tiles are PACKED: their score matmuls write at
                # shifted column offsets so each diagonal mega is one
                # contiguous region -> one exp per mega (2 per chunk, not 4).
                # Matmul moving-operand columns map to output columns by
                # position, so a shifted E slice feeds U/dsum unchanged.
                #   mega D1: t0 at flat [0:512],  t1 at flat [512:896]
                #   mega D2: t2 at flat [0:256],  t3 at flat [256:384]
                # dsum: the 4 ragged diagonal tiles are combined with
                # column-ALIGNED slice adds into one [128,512] quad (Pool),
                # so each chunk needs 1 diagonal d-matmul + 1 per full quad.
                ut = ut_pool.tile([C, M], f32)
                d_sb = d_pool.tile([1, M], f32)
                # task kinds: "D1"/"D2" diagonal megas, "F" full pair
                pairs = []   # (cch, kind, j0, j1, first, last)
                for cch in range(NCHUNK):
                    pl = [("D1", 4 * cch, 4 * cch + 1),
                          ("D2", 4 * cch + 2, 4 * cch + 3)]
                    pl += [("F", 2 * i, 2 * i + 1) for i in range(2 * cch)]
                    for k, (kind, a, b) in enumerate(pl):
                        pairs.append((cch, kind, a, b, k == 0, k == len(pl) - 1))

                n_pairs = len(pairs)
                e_state = {}
                chunk_state = {}

                def emit_score(p):
                    cch, kind, j0, j1, _, _ = pairs[p]
                    q0 = cch * 512
                    sm = psS.tile([128, 2, 512], f32, tag="s", name="sm")
                    e = exp_pool.tile([128, 2, 512], bf16, name="e")
                    ef = e.rearrange("p a b -> p (a b)")
                    smf = sm.rearrange("p a b -> p (a b)")
                    if kind == "D1":
                        # t0: q [0:512) at flat [0:512); t1: q [128:512) at flat [512:896)
                        nc.tensor.matmul(smf[:, 0:512], xt[:, ts(j0, 128)],
                                         pt[:, q0 : q0 + 512])
                        nc.tensor.matmul(smf[:, 512:896], xt[:, ts(j1, 128)],
                                         pt[:, q0 + 128 : q0 + 512])
                        nc.scalar.activation(
                            out=ef[:, 0:896], in_=smf[:, 0:896],
                            func=mybir.ActivationFunctionType.Exp,
                        )
                        nc.gpsimd.tensor_mul(out=ef[:, 0:128],
                                             in0=ef[:, 0:128], in1=msk_sb[:])
                        nc.gpsimd.tensor_mul(out=ef[:, 512:640],
                                             in0=ef[:, 512:640], in1=msk_sb[:])
                    elif kind == "D2":
                        # t2: q [256:512) at flat [0:256); t3: q [384:512) at flat [256:384)
                        nc.tensor.matmul(smf[:, 0:256], xt[:, ts(j0, 128)],
                                         pt[:, q0 + 256 : q0 + 512])
                        nc.tensor.matmul(smf[:, 256:384], xt[:, ts(j1, 128)],
                                         pt[:, q0 + 384 : q0 + 512])
                        nc.scalar.activation(
                            out=ef[:, 0:384], in_=smf[:, 0:384],
                            func=mybir.ActivationFunctionType.Exp,
                        )
                        nc.gpsimd.tensor_mul(out=ef[:, 0:128],
                                             in0=ef[:, 0:128], in1=msk_sb[:])
                        nc.gpsimd.tensor_mul(out=ef[:, 256:384],
                                             in0=ef[:, 256:384], in1=msk_sb[:])
                    else:
                        for h, j in enumerate((j0, j1)):
                            nc.tensor.matmul(sm[:, h, :], xt[:, ts(j, 128)],
                                             pt[:, q0 : q0 + 512])
                        nc.scalar.activation(
                            out=ef[:], in_=smf[:],
                            func=mybir.ActivationFunctionType.Exp,
                        )
                    e_state[p] = e

                def emit_accum(p):
                    cch, kind, j0, j1, first, last = pairs[p]
                    e = e_state.pop(p)
                    ef = e.rearrange("p a b -> p (a b)")
                    if first:
                        chunk_state[cch] = {
                            "u": ps_u_pool.tile([128, 512], f32, name="ps_u"),
                            "d": ps_d_pool.tile([1, 512], f32, name="ps_d"),
                            "done": [],       # (kind, ef) for chunk-end dsum
                        }
                    st = chunk_state[cch]
                    ps_u, ps_d = st["u"], st["d"]
                    if kind == "D1":
                        nc.tensor.matmul(ps_u[:, 0:512], v_sb[:, j0, :],
                                         ef[:, 0:512], start=True, stop=False)
                        nc.tensor.matmul(ps_u[:, 128:512], v_sb[:, j1, :],
                                         ef[:, 512:896], start=False,
                                         stop=False)
                    elif kind == "D2":
                        stop_u = (cch == 0)
                        nc.tensor.matmul(ps_u[:, 256:512], v_sb[:, j0, :],
                                         ef[:, 0:256], start=False, stop=False)
                        nc.tensor.matmul(ps_u[:, 384:512], v_sb[:, j1, :],
                                         ef[:, 256:384], start=False,
                                         stop=stop_u)
                    else:
                        for h, j in enumerate((j0, j1)):
                            nc.tensor.matmul(
                                ps_u[:, 0:512], v_sb[:, j, :], e[:, h, :],
                                start=False, stop=(last and h == 1),
                            )
                    st["done"].append((kind, ef))
                    if last:
                        # dsum: per-tile [1,512] matmuls, all back-to-back so
                        # the `ones` stationary loads once.  Moving-operand
                        # columns map to output columns by position, so the
                        # packed diagonal slices land on their query ranges.
                        dms = []
                        for knd, eff in st["done"]:
                            if knd == "D1":
                                dms += [(0, eff[:, 0:512]), (128, eff[:, 512:896])]
                            elif knd == "D2":
                                dms += [(256, eff[:, 0:256]), (384, eff[:, 256:384])]
                            else:
                                dms += [(0, eff[:, 0:512]), (0, eff[:, 512:1024])]
                        for i, (lo, eap) in enumerate(dms):
                            nc.tensor.matmul(
                                ps_d[:, lo:512], ones_sb[:], eap,
                                start=(i == 0), stop=(i == len(dms) - 1),
                            )
                    if last:
                        # stage through SBUF (DMA cannot read PSUM); u leaves
                        # chunk-wise so the final drain is short, d once/segment
                        nc.vector.tensor_copy(ut[:, ts(cch, 512)], ps_u[:])
                        nc.vector.tensor_copy(d_sb[:, ts(cch, 512)], ps_d[:])
                        nc.sync.dma_start(
                            u_out[s, :, 512 * cch : 512 * (cch + 1)],
                            ut[:, ts(cch, 512)],
                        )

                for p in range(n_pairs + LA):
                    if p < n_pairs:
                        emit_score(p)
                    if p >= LA:
                        emit_accum(p - LA)
                nc.sync.dma_start(d_out[s : s + 1, :], d_sb[:])

    nc.compile()
    return nc


def get_nc(loop_r=None):
    key = ("nc", loop_r)
    if key not in _NC_CACHE:
        _NC_CACHE[key] = _build_nc(loop_r)
    return _NC_CACHE[key]


def _masks():
    """Diagonal-block triangle: msk[kk, qq] = 1.0 iff kk <= qq."""
    kk = np.arange(128)[:, None]
    qq = np.arange(128)[None, :]
    return (kk <= qq).astype(np.float32)


def build_in_maps(x, Wq, Wk, Wv, Wo):
    segs = _segment_list()
    padded = segs + segs[:N_CORES * SEGS_PER_CORE - len(segs)]
    msk = _masks()
    Wq64 = np.asarray(Wq, dtype=np.float64)
    Wk64 = np.asarray(Wk, dtype=np.float64)
    in_maps = []
    for core in range(N_CORES):
        xseg = np.empty((SEGS_PER_CORE, C, M), dtype=np.float32)
        for k in range(SEGS_PER_CORE):
            b, w, r, t = padded[core * SEGS_PER_CORE + k]
            xseg[k] = x[b, t * w + r * np.arange(M), :].T
        in_maps.append({
            "xseg": xseg,
            "xsegh": xseg.astype(np.float16),
            # G = (Wq/sqrt(c)) Wk^T folded on the host: S = X G X^T
            "g": (Wq64 @ Wk64.T).astype(np.float32) * np.float32(SCALE),
            # W2 = Wv @ Wo folded on the host; Wo never ships to the device
            "wv": (np.asarray(Wv, dtype=np.float64) @ np.asarray(Wo, dtype=np.float64)).astype(np.float16),
            "msk": msk,
        })
    return in_maps, padded


def combine(results, padded):
    """results: per-core dicts with u [S,C,M] and d [S,M]."""
    numer = np.zeros((B, N, C), dtype=np.float64)
    den = np.zeros((B, N), dtype=np.float64)
    seen = set()
    for core in range(N_CORES):
        for k in range(SEGS_PER_CORE):
            key = padded[core * SEGS_PER_CORE + k]
            if key in seen:
                continue
            seen.add(key)
            b, w, r, t = key
            pos = t * w + r * np.arange(M)
            numer[b, pos, :] += results[core]["u"][k].T.astype(np.float64)
            den[b, pos] += results[core]["d"][k].astype(np.float64)
    return (numer / den[..., None]).astype(np.float32)


def kernel(x, Wq, Wk, Wv, Wo):
    from concourse.bass_utils import run_bass_kernel_spmd

    x = np.asarray(x, dtype=np.float32)
    nc = get_nc()
    in_maps, padded = build_in_maps(x, Wq, Wk, Wv, Wo)
    res = run_bass_kernel_spmd(nc, in_maps, core_ids=list(range(N_CORES)))
    return combine(res.results, padded)


if __name__ == "__main__":
    rng = np.random.default_rng(0)
    x = rng.standard_normal((B, N, C)).astype(np.float32)
    Wq, Wk, Wv, Wo = [
        (rng.standard_normal((C, C)) / np.sqrt(C)).astype(np.float32)
        for _ in range(4)
    ]
    out = kernel(x, Wq, Wk, Wv, Wo)
    print("out", out.shape, out.dtype, np.abs(out).max())
